# revision 6
# baseline (speedup 1.0000x reference)
"""Trainium2 Bass kernel for nn_MultiHeadAttention_88338887344199.

Head-sharded (tensor-parallel) multi-head attention across 8 NeuronCores:
  - each core owns 2 of the 16 heads: computes Q/K/V projections, scores,
    softmax over the QUERY axis (the reference's faithful quirk), attn @ V,
    and writes its heads' full (q, m) attention matrices.
  - head outputs are exchanged with an AllToAll so each core ends up with
    the full concatenated head output for its 256-row slice of the sequence,
    then computes the output projection + residual + LayerNorm for that slice.
  - host side only shards/reassembles (transpose of activations is host-side
    input prep; all math runs on device).

Shapes are hardcoded per the problem spec:
  L = 2048 tokens, D = 2048 model dim, H = 16 heads, dk = dv = 128.
"""

import os
import sys

import numpy as np

_AXON_PATHS = [
    "/root/.axon_site",
    "/root/.axon_site/_ro/trn_rl_repo",
    "/root/.axon_site/_ro/pypackages",
]
for _p in _AXON_PATHS:
    if os.path.isdir(_p) and _p not in sys.path:
        sys.path.append(_p)

import concourse.bass as bass  # noqa: E402
import concourse.mybir as mybir  # noqa: E402
import concourse.tile as tile  # noqa: E402
from concourse import bacc  # noqa: E402
from concourse.bass_utils import run_bass_kernel_spmd  # noqa: E402
from concourse.masks import make_identity  # noqa: E402

F32 = mybir.dt.float32
F32R = mybir.dt.float32r

L = 2048          # sequence length
D = 2048          # model dim
H = 16            # heads (global)
DK = 128          # head dim
P = 128           # partitions
NCORES = 8
HPC = H // NCORES          # heads per core = 2
ROWS = L // NCORES         # output rows per core = 256
NDC = D // P               # contraction chunks = 16
NMC = L // P               # m (key) chunks = 16
NQ4 = L // 512             # 512-wide q/d chunks = 4
INV_TEMPER = float(1.0 / np.sqrt(np.float64(D)))
LN_EPS = 1e-5


def r32(ap):
    return ap.bitcast(F32R)


def build_nc():
    nc = bacc.Bacc("TRN2", target_bir_lowering=False, num_devices=NCORES)

    # ---- I/O ----
    qT = nc.dram_tensor("qT", [D, L], F32R, kind="ExternalInput")
    kT = nc.dram_tensor("kT", [D, L], F32R, kind="ExternalInput")
    vT = nc.dram_tensor("vT", [D, L], F32R, kind="ExternalInput")
    wq = nc.dram_tensor("wq", [D, HPC * DK], F32R, kind="ExternalInput")
    wk = nc.dram_tensor("wk", [D, HPC * DK], F32R, kind="ExternalInput")
    wv = nc.dram_tensor("wv", [D, HPC * DK], F32R, kind="ExternalInput")
    pw = nc.dram_tensor("pw", [D, D], F32R, kind="ExternalInput")
    qrows = nc.dram_tensor("qrows", [ROWS, D], F32, kind="ExternalInput")
    pb = nc.dram_tensor("pb", [1, D], F32, kind="ExternalInput")
    lg = nc.dram_tensor("lg", [1, D], F32, kind="ExternalInput")
    lb = nc.dram_tensor("lb", [1, D], F32, kind="ExternalInput")

    # attn output as (head, qchunk, p, m); host reshapes to (2, 2048, 2048)
    attn_out = nc.dram_tensor("attn_out", [HPC, NMC, P, L], F32, kind="ExternalOutput")
    y_out = nc.dram_tensor("y_out", [ROWS, D], F32, kind="ExternalOutput")

    with tile.TileContext(nc) as tc:
        _build_tile_program(nc, tc, qT, kT, vT, wq, wk, wv, pw, qrows,
                            pb, lg, lb, attn_out, y_out)
    nc.compile()
    return nc


def _build_tile_program(nc, tc, qT, kT, vT, wq, wk, wv, pw, qrows,
                        pb, lg, lb, attn_out, y_out):
    from contextlib import ExitStack

    with ExitStack() as ctx:
        const = ctx.enter_context(tc.tile_pool(name="const", bufs=1))
        persist = ctx.enter_context(tc.tile_pool(name="persist", bufs=1))
        dram = ctx.enter_context(tc.tile_pool(name="dram", bufs=1, space="DRAM"))

        identity = const.tile([P, P], F32)
        make_identity(nc, identity[:])
        identity_r = const.tile([P, P], F32R)
        nc.vector.tensor_copy(identity_r[:], identity[:])

        # persistent per-head projections
        qhT = persist.tile([P, HPC, L], F32R)   # (dk, head, L)
        khT = persist.tile([P, HPC, L], F32R)   # (dk, head, L)
        vh = persist.tile([P, NMC, HPC * DK], F32R)   # (m_in_chunk, mchunk, h*dv)
        outT = persist.tile([P, HPC, L], F32R)  # (dv, head, q)

        # A2A buffers: in is (destcore j, head h, p, c) == (2048, 256) flat;
        # out is (hdv chunk, p, c) == (2048, 256) flat.
        a2a_in = dram.tile([NCORES, HPC, P, ROWS], F32R)
        a2a_out = dram.tile([NDC, P, ROWS], F32R)

        # ---------------- Phase 1: projections ----------------
        # qhT/khT/vhT: out = W[dc].T @ actT[dc] accumulated over dc.
        with tc.tile_pool(name="vhT_pool", bufs=1) as vhT_pool, \
             tc.tile_pool(name="acts", bufs=3) as acts, \
             tc.tile_pool(name="wts", bufs=3) as wts:
            vhT = vhT_pool.tile([P, HPC, L], F32)
            with tc.tile_pool(name="ps1", bufs=2, space="PSUM") as ps1:
                for actT_d, w_d, dstT in ((qT, wq, qhT), (kT, wk, khT), (vT, wv, vhT)):
                    ps_h = [ps1.tile([P, L], F32, name="ps1t", tag="ps1")
                            for _ in range(HPC)]
                    for dc in range(NDC):
                        at = acts.tile([P, L], F32R)
                        nc.sync.dma_start(at[:], actT_d[dc * P:(dc + 1) * P, :])
                        wt = wts.tile([P, HPC * DK], F32R)
                        nc.sync.dma_start(wt[:], w_d[dc * P:(dc + 1) * P, :])
                        for h in range(HPC):
                            for j in range(NQ4):
                                nc.tensor.matmul(
                                    ps_h[h][:, j * 512:(j + 1) * 512],
                                    wt[:, h * DK:(h + 1) * DK],
                                    at[:, j * 512:(j + 1) * 512],
                                    start=(dc == 0), stop=(dc == NDC - 1),
                                )
                    for h in range(HPC):
                        nc.vector.tensor_copy(dstT[:, h, :], ps_h[h][:])

            # vh = transpose(vhT): (dv, m) -> (m, dv) blocks
            with tc.tile_pool(name="psTr1", bufs=2, space="PSUM") as psTr1:
                for h in range(HPC):
                    for mc in range(NMC):
                        tp = psTr1.tile([P, P], F32)
                        nc.tensor.transpose(
                            tp[:], vhT[:, h, mc * P:(mc + 1) * P], identity[:])
                        nc.vector.tensor_copy(vh[:, mc, h * DK:(h + 1) * DK], tp[:])

        # ---------------- Phase 2+3: scores/softmax/attn, attn @ V ----------
        # per (head, mchunk):
        #   scoresT (m, q) = khT[:, m]T @ qhT  (single K=128 matmul per tile)
        #   attn_raw = exp(scoresT / temper)  with per-row (per-m) sums
        #   attn = attn_raw * (1/sum)         [softmax over q for fixed m]
        #   outT += vh[mc].T @ attn           (accumulate over mchunks)
        #   attn.T blocks -> (q, m) layout -> DRAM
        with tc.tile_pool(name="attn_sb", bufs=3) as attn_sb_pool, \
             tc.tile_pool(name="asm", bufs=2) as asm_pool, \
             tc.tile_pool(name="ssum", bufs=8) as ssum_pool, \
             tc.tile_pool(name="psS", bufs=2, space="PSUM") as psS_pool, \
             tc.tile_pool(name="psO", bufs=1, space="PSUM") as psO_pool, \
             tc.tile_pool(name="psT", bufs=2, space="PSUM") as psT_pool:
            for h in range(HPC):
                ps_o = psO_pool.tile([P, L], F32)
                for mc in range(NMC):
                    attn_raw = attn_sb_pool.tile([P, L], F32, tag="attn_raw")
                    attn_s = attn_sb_pool.tile([P, L], F32R, tag="attn_s")
                    ssums = []
                    for j in range(NQ4):
                        ps_s = psS_pool.tile([P, 512], F32)
                        nc.tensor.matmul(
                            ps_s[:],
                            khT[:, h, mc * P:(mc + 1) * P],
                            qhT[:, h, j * 512:(j + 1) * 512],
                            start=True, stop=True,
                        )
                        ssum = ssum_pool.tile([P, 1], F32)
                        nc.scalar.activation(
                            out=attn_raw[:, j * 512:(j + 1) * 512],
                            in_=ps_s[:],
                            func=mybir.ActivationFunctionType.Exp,
                            scale=INV_TEMPER,
                            accum_out=ssum[:],
                        )
                        ssums.append(ssum)
                    s01 = ssum_pool.tile([P, 1], F32, tag="s01")
                    s23 = ssum_pool.tile([P, 1], F32, tag="s23")
                    stot = ssum_pool.tile([P, 1], F32, tag="stot")
                    rs = ssum_pool.tile([P, 1], F32, tag="rs")
                    nc.vector.tensor_add(s01[:], ssums[0][:], ssums[1][:])
                    nc.vector.tensor_add(s23[:], ssums[2][:], ssums[3][:])
                    nc.vector.tensor_add(stot[:], s01[:], s23[:])
                    nc.vector.reciprocal(rs[:], stot[:])
                    # normalize: attn = attn_raw * r  (r per-partition = per-m)
                    nc.vector.tensor_scalar_mul(attn_s[:], attn_raw[:], rs[:])

                    # attn @ V accumulation: outT[dv, q] += vh[mc,h].T @ attn
                    for j in range(NQ4):
                        nc.tensor.matmul(
                            ps_o[:, j * 512:(j + 1) * 512],
                            vh[:, mc, h * DK:(h + 1) * DK],
                            attn_s[:, j * 512:(j + 1) * 512],
                            start=(mc == 0), stop=(mc == NMC - 1),
                        )

                    # transpose attn tiles to (q, m) layout and write out
                    asm = asm_pool.tile([P, NMC, P], F32)
                    for qc in range(NMC):
                        tp = psT_pool.tile([P, P], F32R, tag="tp23")
                        nc.tensor.transpose(
                            tp[:], attn_s[:, qc * P:(qc + 1) * P], identity_r[:])
                        if qc % 2 == 0:
                            nc.vector.tensor_copy(asm[:, qc, :], tp[:])
                        else:
                            nc.scalar.activation(
                                out=asm[:, qc, :], in_=tp[:],
                                func=mybir.ActivationFunctionType.Copy)
                    nc.scalar.dma_start(
                        attn_out[h].rearrange("j p m -> p j m")[:, :, mc * P:(mc + 1) * P],
                        asm[:],
                    )
                nc.vector.tensor_copy(outT[:, h, :], ps_o[:])

        # ---------------- Phase 4: AllToAll of head outputs ------------------
        for h in range(HPC):
            nc.scalar.dma_start(
                a2a_in[:, h, :, :].rearrange("j p c -> p j c"),
                outT[:, h, :].rearrange("p (j c) -> p j c", c=ROWS),
            )
        nc.gpsimd.collective_compute(
            "AllToAll",
            mybir.AluOpType.bypass,
            replica_groups=[list(range(NCORES))],
            ins=[a2a_in.opt()],
            outs=[a2a_out.opt()],
        )

        # ---------------- Phase 5: output projection -------------------------
        # proj[q, d] = sum_hdv out_cat[q, hdv] * pw[hdv, d] for q in our rows
        with tc.tile_pool(name="pwp", bufs=3) as pwp, \
             tc.tile_pool(name="a2t", bufs=3) as a2tp, \
             tc.tile_pool(name="psY", bufs=2, space="PSUM") as psY_pool, \
             tc.tile_pool(name="ln", bufs=1) as lnp, \
             tc.tile_pool(name="lnt", bufs=2) as lntp:
            ps_y = [psY_pool.tile([P, D], F32, name="psYt", tag="psY") for _ in range(ROWS // P)]
            for hc in range(NDC):
                a2t = a2tp.tile([P, ROWS], F32R)
                nc.sync.dma_start(a2t[:], a2a_out[hc])
                pwt = pwp.tile([P, D], F32R)
                nc.sync.dma_start(pwt[:], pw[hc * P:(hc + 1) * P, :])
                for qc in range(ROWS // P):
                    for j in range(NQ4):
                        nc.tensor.matmul(
                            ps_y[qc][:, j * 512:(j + 1) * 512],
                            a2t[:, qc * P:(qc + 1) * P],
                            pwt[:, j * 512:(j + 1) * 512],
                            start=(hc == 0), stop=(hc == NDC - 1),
                        )

            # ------------- Phase 6: +bias +residual, LayerNorm, store -------
            pbb = lnp.tile([P, D], F32)
            lgb = lnp.tile([P, D], F32)
            lbb = lnp.tile([P, D], F32)
            for t, d in ((pbb, pb), (lgb, lg), (lbb, lb)):
                bcast = bass.AP(tensor=d.ap().tensor, offset=0,
                                ap=[[0, P], [1, D]])
                nc.gpsimd.dma_start(t[:], bcast)
            eps_t = lnp.tile([P, 1], F32)
            nc.vector.memset(eps_t[:], LN_EPS)

            fmax = nc.vector.BN_STATS_FMAX
            nsub = D // fmax
            for qc in range(ROWS // P):
                qr_t = lntp.tile([P, D], F32, tag="qr")
                nc.sync.dma_start(qr_t[:], qrows[qc * P:(qc + 1) * P, :])
                y0 = lntp.tile([P, D], F32, tag="y0")
                nc.vector.tensor_add(y0[:], ps_y[qc][:], qr_t[:])
                nc.vector.tensor_add(y0[:], y0[:], pbb[:])

                stats = lntp.tile([P, nsub, nc.vector.BN_STATS_DIM], F32, tag="st")
                for sg in range(nsub):
                    nc.vector.bn_stats(
                        out=stats[:, sg, :],
                        in_=y0[:, sg * fmax:(sg + 1) * fmax])
                mv = lntp.tile([P, nc.vector.BN_AGGR_DIM], F32, tag="mv")
                nc.vector.bn_aggr(out=mv[:], in_=stats[:])
                mean = mv[:, 0:1]
                rstd = mv[:, 1:2]
                nc.scalar.activation(
                    out=rstd, in_=rstd,
                    func=mybir.ActivationFunctionType.Sqrt,
                    bias=eps_t[:], scale=1.0)
                nc.vector.reciprocal(out=rstd, in_=rstd)
                nc.vector.tensor_scalar(
                    out=y0[:], in0=y0[:],
                    scalar1=mean, scalar2=rstd,
                    op0=mybir.AluOpType.subtract, op1=mybir.AluOpType.mult)
                nc.vector.tensor_mul(y0[:], y0[:], lgb[:])
                nc.vector.tensor_add(y0[:], y0[:], lbb[:])
                nc.sync.dma_start(y_out[qc * P:(qc + 1) * P, :], y0[:])


_NC_CACHE = None


def _get_nc():
    global _NC_CACHE
    if _NC_CACHE is None:
        _NC_CACHE = build_nc()
    return _NC_CACHE


def shard_inputs(q, k, v, Wq, Wk, Wv, proj_w, proj_b, ln_g, ln_b):
    f = np.float32
    qs, ks, vs = (np.asarray(x, f)[0] for x in (q, k, v))
    qT = np.ascontiguousarray(qs.T)
    kT = np.ascontiguousarray(ks.T)
    vT = np.ascontiguousarray(vs.T)
    pw = np.ascontiguousarray(np.asarray(proj_w, f))
    pb = np.ascontiguousarray(np.asarray(proj_b, f).reshape(1, D))
    lg = np.ascontiguousarray(np.asarray(ln_g, f).reshape(1, D))
    lb = np.ascontiguousarray(np.asarray(ln_b, f).reshape(1, D))
    Wq, Wk, Wv = (np.asarray(x, f) for x in (Wq, Wk, Wv))

    in_maps = []
    for c in range(NCORES):
        h0 = HPC * c
        in_maps.append({
            "qT": qT, "kT": kT, "vT": vT,
            "wq": np.ascontiguousarray(np.concatenate([Wq[h0 + i] for i in range(HPC)], axis=1)),
            "wk": np.ascontiguousarray(np.concatenate([Wk[h0 + i] for i in range(HPC)], axis=1)),
            "wv": np.ascontiguousarray(np.concatenate([Wv[h0 + i] for i in range(HPC)], axis=1)),
            "pw": pw,
            "qrows": np.ascontiguousarray(qs[ROWS * c:ROWS * (c + 1)]),
            "pb": pb, "lg": lg, "lb": lb,
        })
    return in_maps


def assemble_outputs(results):
    y = np.concatenate([results[c]["y_out"] for c in range(NCORES)], axis=0)[None]
    attn = np.concatenate(
        [results[c]["attn_out"].reshape(HPC, L, L) for c in range(NCORES)], axis=0)
    return y.astype(np.float32, copy=False), attn.astype(np.float32, copy=False)


def run_sharded(in_maps, trace=False):
    nc = _get_nc()
    res = run_bass_kernel_spmd(nc, in_maps, core_ids=list(range(NCORES)),
                               trace=trace)
    return res


def kernel(**inputs):
    in_maps = shard_inputs(**inputs)
    res = run_sharded(in_maps, trace=False)
    return assemble_outputs(res.results)


# revision 7
# speedup vs baseline: 1.3743x; 1.3743x over previous
"""Trainium2 Bass kernel for nn_MultiHeadAttention_88338887344199.

Head-sharded (tensor-parallel) multi-head attention across 8 NeuronCores:
  - each core owns 2 of the 16 heads: computes Q/K/V projections, scores,
    softmax over the QUERY axis (the reference's faithful quirk), attn @ V,
    and writes its heads' full (q, m) attention matrices.
  - head outputs are exchanged with an AllToAll so each core ends up with
    the full concatenated head output for its 256-row slice of the sequence,
    then computes the output projection + residual + LayerNorm for that slice.
  - host side only shards/reassembles (transpose of activations is host-side
    input prep; all math runs on device).

Shapes are hardcoded per the problem spec:
  L = 2048 tokens, D = 2048 model dim, H = 16 heads, dk = dv = 128.
"""

import os
import sys

import numpy as np

_AXON_PATHS = [
    "/root/.axon_site",
    "/root/.axon_site/_ro/trn_rl_repo",
    "/root/.axon_site/_ro/pypackages",
]
for _p in _AXON_PATHS:
    if os.path.isdir(_p) and _p not in sys.path:
        sys.path.append(_p)

import concourse.bass as bass  # noqa: E402
import concourse.mybir as mybir  # noqa: E402
import concourse.tile as tile  # noqa: E402
from concourse import bacc  # noqa: E402
from concourse.bass_utils import run_bass_kernel_spmd  # noqa: E402
from concourse.masks import make_identity  # noqa: E402

F32 = mybir.dt.float32
F32R = mybir.dt.float32r

L = 2048          # sequence length
D = 2048          # model dim
H = 16            # heads (global)
DK = 128          # head dim
P = 128           # partitions
NCORES = 8
HPC = H // NCORES          # heads per core = 2
ROWS = L // NCORES         # output rows per core = 256
NDC = D // P               # contraction chunks = 16
NMC = L // P               # m (key) chunks = 16
NQ4 = L // 512             # 512-wide q/d chunks = 4
INV_TEMPER = float(1.0 / np.sqrt(np.float64(D)))
LN_EPS = 1e-5


def r32(ap):
    return ap.bitcast(F32R)


def build_nc():
    nc = bacc.Bacc("TRN2", target_bir_lowering=False, num_devices=NCORES)

    # ---- I/O ----
    qT = nc.dram_tensor("qT", [D, L], F32R, kind="ExternalInput")
    kT = nc.dram_tensor("kT", [D, L], F32R, kind="ExternalInput")
    vT = nc.dram_tensor("vT", [D, L], F32R, kind="ExternalInput")
    wq = nc.dram_tensor("wq", [D, HPC * DK], F32R, kind="ExternalInput")
    wk = nc.dram_tensor("wk", [D, HPC * DK], F32R, kind="ExternalInput")
    wv = nc.dram_tensor("wv", [D, HPC * DK], F32R, kind="ExternalInput")
    pw = nc.dram_tensor("pw", [D, D], F32R, kind="ExternalInput")
    qrows = nc.dram_tensor("qrows", [ROWS, D], F32, kind="ExternalInput")
    pb = nc.dram_tensor("pb", [1, D], F32, kind="ExternalInput")
    lg = nc.dram_tensor("lg", [1, D], F32, kind="ExternalInput")
    lb = nc.dram_tensor("lb", [1, D], F32, kind="ExternalInput")

    # attn output in transposed (head, m, q) layout; host transposes to (q, m)
    attn_out = nc.dram_tensor("attn_out", [HPC, L, L], F32, kind="ExternalOutput")
    y_out = nc.dram_tensor("y_out", [ROWS, D], F32, kind="ExternalOutput")

    with tile.TileContext(nc) as tc:
        _build_tile_program(nc, tc, qT, kT, vT, wq, wk, wv, pw, qrows,
                            pb, lg, lb, attn_out, y_out)
    nc.compile()
    return nc


def _build_tile_program(nc, tc, qT, kT, vT, wq, wk, wv, pw, qrows,
                        pb, lg, lb, attn_out, y_out):
    from contextlib import ExitStack

    with ExitStack() as ctx:
        const = ctx.enter_context(tc.tile_pool(name="const", bufs=1))
        persist = ctx.enter_context(tc.tile_pool(name="persist", bufs=1))
        dram = ctx.enter_context(tc.tile_pool(name="dram", bufs=1, space="DRAM"))

        identity = const.tile([P, P], F32)
        make_identity(nc, identity[:])
        identity_r = const.tile([P, P], F32R)
        nc.vector.tensor_copy(identity_r[:], identity[:])

        # persistent per-head projections
        qhT = persist.tile([P, HPC, L], F32R)   # (dk, head, L)
        khT = persist.tile([P, HPC, L], F32R)   # (dk, head, L)
        vh = persist.tile([P, NMC, HPC * DK], F32R)   # (m_in_chunk, mchunk, h*dv)
        outT = persist.tile([P, HPC, L], F32R)  # (dv, head, q)

        # A2A buffers: in is (destcore j, head h, p, c) == (2048, 256) flat;
        # out is (hdv chunk, p, c) == (2048, 256) flat.
        a2a_in = dram.tile([NCORES, HPC, P, ROWS], F32R)
        a2a_out = dram.tile([NDC, P, ROWS], F32R)

        # ---------------- Phase 1: projections ----------------
        # qhT/khT/vhT: out = W[dc].T @ actT[dc] accumulated over dc.
        with tc.tile_pool(name="vhT_pool", bufs=1) as vhT_pool, \
             tc.tile_pool(name="acts", bufs=3) as acts, \
             tc.tile_pool(name="wts", bufs=3) as wts:
            vhT = vhT_pool.tile([P, HPC, L], F32)
            with tc.tile_pool(name="ps1", bufs=2, space="PSUM") as ps1:
                for actT_d, w_d, dstT in ((qT, wq, qhT), (kT, wk, khT), (vT, wv, vhT)):
                    ps_h = [ps1.tile([P, L], F32, name="ps1t", tag="ps1")
                            for _ in range(HPC)]
                    for dc in range(NDC):
                        at = acts.tile([P, L], F32R)
                        nc.sync.dma_start(at[:], actT_d[dc * P:(dc + 1) * P, :])
                        wt = wts.tile([P, HPC * DK], F32R)
                        nc.sync.dma_start(wt[:], w_d[dc * P:(dc + 1) * P, :])
                        for h in range(HPC):
                            for j in range(NQ4):
                                nc.tensor.matmul(
                                    ps_h[h][:, j * 512:(j + 1) * 512],
                                    wt[:, h * DK:(h + 1) * DK],
                                    at[:, j * 512:(j + 1) * 512],
                                    start=(dc == 0), stop=(dc == NDC - 1),
                                )
                    for h in range(HPC):
                        nc.vector.tensor_copy(dstT[:, h, :], ps_h[h][:])

            # vh = transpose(vhT): (dv, m) -> (m, dv) blocks
            with tc.tile_pool(name="psTr1", bufs=2, space="PSUM") as psTr1:
                for h in range(HPC):
                    for mc in range(NMC):
                        tp = psTr1.tile([P, P], F32)
                        nc.tensor.transpose(
                            tp[:], vhT[:, h, mc * P:(mc + 1) * P], identity[:])
                        nc.vector.tensor_copy(vh[:, mc, h * DK:(h + 1) * DK], tp[:])

        # ---------------- Phase 2+3: scores/softmax/attn, attn @ V ----------
        # per (head, mchunk):
        #   scoresT (m, q) = khT[:, m]T @ qhT  (single K=128 matmul per tile)
        #   attn_raw = exp(scoresT / temper)  with per-row (per-m) sums
        #   attn = attn_raw * (1/sum)         [softmax over q for fixed m]
        #   outT += vh[mc].T @ attn           (accumulate over mchunks)
        #   attn.T blocks -> (q, m) layout -> DRAM
        with tc.tile_pool(name="attn_sb", bufs=3) as attn_sb_pool, \
             tc.tile_pool(name="vhs_sb", bufs=3) as vhs_pool, \
             tc.tile_pool(name="ssum", bufs=8) as ssum_pool, \
             tc.tile_pool(name="psS", bufs=4, space="PSUM") as psS_pool, \
             tc.tile_pool(name="psO", bufs=1, space="PSUM") as psO_pool:
            for h in range(HPC):
                ps_o = psO_pool.tile([P, L], F32, name="ps_o", tag="ps_o")
                for mc in range(NMC):
                    attn_raw = attn_sb_pool.tile([P, L], F32R, tag="attn_raw")
                    attn_s = attn_sb_pool.tile([P, L], F32, tag="attn_s")
                    ssums = []
                    for j in range(NQ4):
                        ps_s = psS_pool.tile([P, 512], F32, name="ps_s", tag="ps_s")
                        nc.tensor.matmul(
                            ps_s[:],
                            khT[:, h, mc * P:(mc + 1) * P],
                            qhT[:, h, j * 512:(j + 1) * 512],
                            start=True, stop=True,
                        )
                        ssum = ssum_pool.tile([P, 1], F32, name="ssum", tag="ssum")
                        nc.scalar.activation(
                            out=attn_raw[:, j * 512:(j + 1) * 512],
                            in_=ps_s[:],
                            func=mybir.ActivationFunctionType.Exp,
                            scale=INV_TEMPER,
                            accum_out=ssum[:],
                        )
                        ssums.append(ssum)
                    s01 = ssum_pool.tile([P, 1], F32, tag="s01")
                    s23 = ssum_pool.tile([P, 1], F32, tag="s23")
                    stot = ssum_pool.tile([P, 1], F32, tag="stot")
                    rs = ssum_pool.tile([P, 1], F32, tag="rs")
                    nc.vector.tensor_add(s01[:], ssums[0][:], ssums[1][:])
                    nc.vector.tensor_add(s23[:], ssums[2][:], ssums[3][:])
                    nc.vector.tensor_add(stot[:], s01[:], s23[:])
                    nc.vector.reciprocal(rs[:], stot[:])

                    # fold 1/sum into the tiny vh slice so the attn@V matmuls
                    # only wait on the (cheap) sums, not the big normalize
                    vhs = vhs_pool.tile([P, DK], F32R, tag="vhs")
                    nc.vector.tensor_scalar_mul(
                        vhs[:], vh[:, mc, h * DK:(h + 1) * DK], rs[:])
                    for j in range(NQ4):
                        nc.tensor.matmul(
                            ps_o[:, j * 512:(j + 1) * 512],
                            vhs[:],
                            attn_raw[:, j * 512:(j + 1) * 512],
                            start=(mc == 0), stop=(mc == NMC - 1),
                        )

                    # normalized attention row block (m, q) -> DRAM (host
                    # transposes to (q, m) during unshard)
                    nc.vector.tensor_scalar_mul(attn_s[:], attn_raw[:], rs[:])
                    nc.scalar.dma_start(
                        attn_out[h, mc * P:(mc + 1) * P, :], attn_s[:])
                nc.vector.tensor_copy(outT[:, h, :], ps_o[:])

        # ---------------- Phase 4: AllToAll of head outputs ------------------
        for h in range(HPC):
            nc.scalar.dma_start(
                a2a_in[:, h, :, :].rearrange("j p c -> p j c"),
                outT[:, h, :].rearrange("p (j c) -> p j c", c=ROWS),
            )
        nc.gpsimd.collective_compute(
            "AllToAll",
            mybir.AluOpType.bypass,
            replica_groups=[list(range(NCORES))],
            ins=[a2a_in.opt()],
            outs=[a2a_out.opt()],
        )

        # ---------------- Phase 5: output projection -------------------------
        # proj[q, d] = sum_hdv out_cat[q, hdv] * pw[hdv, d] for q in our rows
        with tc.tile_pool(name="pwp", bufs=3) as pwp, \
             tc.tile_pool(name="a2t", bufs=3) as a2tp, \
             tc.tile_pool(name="psY", bufs=2, space="PSUM") as psY_pool, \
             tc.tile_pool(name="ln", bufs=1) as lnp, \
             tc.tile_pool(name="lnt", bufs=2) as lntp:
            ps_y = [psY_pool.tile([P, D], F32, name="psYt", tag="psY") for _ in range(ROWS // P)]
            for hc in range(NDC):
                a2t = a2tp.tile([P, ROWS], F32R)
                nc.sync.dma_start(a2t[:], a2a_out[hc])
                pwt = pwp.tile([P, D], F32R)
                nc.sync.dma_start(pwt[:], pw[hc * P:(hc + 1) * P, :])
                for qc in range(ROWS // P):
                    for j in range(NQ4):
                        nc.tensor.matmul(
                            ps_y[qc][:, j * 512:(j + 1) * 512],
                            a2t[:, qc * P:(qc + 1) * P],
                            pwt[:, j * 512:(j + 1) * 512],
                            start=(hc == 0), stop=(hc == NDC - 1),
                        )

            # ------------- Phase 6: +bias +residual, LayerNorm, store -------
            pbb = lnp.tile([P, D], F32)
            lgb = lnp.tile([P, D], F32)
            lbb = lnp.tile([P, D], F32)
            for t, d in ((pbb, pb), (lgb, lg), (lbb, lb)):
                bcast = bass.AP(tensor=d.ap().tensor, offset=0,
                                ap=[[0, P], [1, D]])
                nc.gpsimd.dma_start(t[:], bcast)
            eps_t = lnp.tile([P, 1], F32)
            nc.vector.memset(eps_t[:], LN_EPS)

            fmax = nc.vector.BN_STATS_FMAX
            nsub = D // fmax
            for qc in range(ROWS // P):
                qr_t = lntp.tile([P, D], F32, tag="qr")
                nc.sync.dma_start(qr_t[:], qrows[qc * P:(qc + 1) * P, :])
                y0 = lntp.tile([P, D], F32, tag="y0")
                nc.vector.tensor_add(y0[:], ps_y[qc][:], qr_t[:])
                nc.vector.tensor_add(y0[:], y0[:], pbb[:])

                stats = lntp.tile([P, nsub, nc.vector.BN_STATS_DIM], F32, tag="st")
                for sg in range(nsub):
                    nc.vector.bn_stats(
                        out=stats[:, sg, :],
                        in_=y0[:, sg * fmax:(sg + 1) * fmax])
                mv = lntp.tile([P, nc.vector.BN_AGGR_DIM], F32, tag="mv")
                nc.vector.bn_aggr(out=mv[:], in_=stats[:])
                mean = mv[:, 0:1]
                rstd = mv[:, 1:2]
                nc.scalar.activation(
                    out=rstd, in_=rstd,
                    func=mybir.ActivationFunctionType.Sqrt,
                    bias=eps_t[:], scale=1.0)
                nc.vector.reciprocal(out=rstd, in_=rstd)
                nc.vector.tensor_scalar(
                    out=y0[:], in0=y0[:],
                    scalar1=mean, scalar2=rstd,
                    op0=mybir.AluOpType.subtract, op1=mybir.AluOpType.mult)
                nc.vector.tensor_mul(y0[:], y0[:], lgb[:])
                nc.vector.tensor_add(y0[:], y0[:], lbb[:])
                nc.sync.dma_start(y_out[qc * P:(qc + 1) * P, :], y0[:])


_NC_CACHE = None


def _get_nc():
    global _NC_CACHE
    if _NC_CACHE is None:
        _NC_CACHE = build_nc()
    return _NC_CACHE


def shard_inputs(q, k, v, Wq, Wk, Wv, proj_w, proj_b, ln_g, ln_b):
    f = np.float32
    qs, ks, vs = (np.asarray(x, f)[0] for x in (q, k, v))
    qT = np.ascontiguousarray(qs.T)
    kT = np.ascontiguousarray(ks.T)
    vT = np.ascontiguousarray(vs.T)
    pw = np.ascontiguousarray(np.asarray(proj_w, f))
    pb = np.ascontiguousarray(np.asarray(proj_b, f).reshape(1, D))
    lg = np.ascontiguousarray(np.asarray(ln_g, f).reshape(1, D))
    lb = np.ascontiguousarray(np.asarray(ln_b, f).reshape(1, D))
    Wq, Wk, Wv = (np.asarray(x, f) for x in (Wq, Wk, Wv))

    in_maps = []
    for c in range(NCORES):
        h0 = HPC * c
        in_maps.append({
            "qT": qT, "kT": kT, "vT": vT,
            "wq": np.ascontiguousarray(np.concatenate([Wq[h0 + i] for i in range(HPC)], axis=1)),
            "wk": np.ascontiguousarray(np.concatenate([Wk[h0 + i] for i in range(HPC)], axis=1)),
            "wv": np.ascontiguousarray(np.concatenate([Wv[h0 + i] for i in range(HPC)], axis=1)),
            "pw": pw,
            "qrows": np.ascontiguousarray(qs[ROWS * c:ROWS * (c + 1)]),
            "pb": pb, "lg": lg, "lb": lb,
        })
    return in_maps


def assemble_outputs(results):
    y = np.concatenate([results[c]["y_out"] for c in range(NCORES)], axis=0)[None]
    attn = np.empty((H, L, L), np.float32)
    for c in range(NCORES):
        a = results[c]["attn_out"]          # (HPC, m, q) transposed layout
        for h in range(HPC):
            np.copyto(attn[HPC * c + h], a[h].T)
    return y.astype(np.float32, copy=False), attn


def run_sharded(in_maps, trace=False):
    nc = _get_nc()
    res = run_bass_kernel_spmd(nc, in_maps, core_ids=list(range(NCORES)),
                               trace=trace)
    return res


def kernel(**inputs):
    in_maps = shard_inputs(**inputs)
    res = run_sharded(in_maps, trace=False)
    return assemble_outputs(res.results)


# revision 9
# speedup vs baseline: 1.6586x; 1.2069x over previous
"""Trainium2 Bass kernel for nn_MultiHeadAttention_88338887344199.

Head-sharded (tensor-parallel) multi-head attention across 8 NeuronCores:
  - each core owns 2 of the 16 heads: computes Q/K/V projections, scores,
    softmax over the QUERY axis (the reference's faithful quirk), attn @ V,
    and writes its heads' full (q, m) attention matrices.
  - head outputs are exchanged with an AllToAll so each core ends up with
    the full concatenated head output for its 256-row slice of the sequence,
    then computes the output projection + residual + LayerNorm for that slice.
  - host side only shards/reassembles (transpose of activations is host-side
    input prep; all math runs on device).

Shapes are hardcoded per the problem spec:
  L = 2048 tokens, D = 2048 model dim, H = 16 heads, dk = dv = 128.
"""

import os
import sys

import numpy as np

_AXON_PATHS = [
    "/root/.axon_site",
    "/root/.axon_site/_ro/trn_rl_repo",
    "/root/.axon_site/_ro/pypackages",
]
for _p in _AXON_PATHS:
    if os.path.isdir(_p) and _p not in sys.path:
        sys.path.append(_p)

import concourse.bass as bass  # noqa: E402
import concourse.mybir as mybir  # noqa: E402
import concourse.tile as tile  # noqa: E402
from concourse import bacc  # noqa: E402
from concourse.bass_utils import run_bass_kernel_spmd  # noqa: E402
from concourse.masks import make_identity  # noqa: E402

F32 = mybir.dt.float32
F32R = mybir.dt.float32r
BF16 = mybir.dt.bfloat16

L = 2048          # sequence length
D = 2048          # model dim
H = 16            # heads (global)
DK = 128          # head dim
P = 128           # partitions
NCORES = 8
HPC = H // NCORES          # heads per core = 2
ROWS = L // NCORES         # output rows per core = 256
NDC = D // P               # contraction chunks = 16
NMC = L // P               # m (key) chunks = 16
NQ4 = L // 512             # 512-wide q/d chunks = 4
INV_TEMPER = float(1.0 / np.sqrt(np.float64(D)))
LN_EPS = 1e-5


def r32(ap):
    return ap.bitcast(F32R)


def build_nc():
    nc = bacc.Bacc("TRN2", target_bir_lowering=False, num_devices=NCORES)

    # ---- I/O ----
    qT = nc.dram_tensor("qT", [D, L], BF16, kind="ExternalInput")
    kT = nc.dram_tensor("kT", [D, L], BF16, kind="ExternalInput")
    vT = nc.dram_tensor("vT", [D, L], BF16, kind="ExternalInput")
    wq = nc.dram_tensor("wq", [D, HPC * DK], BF16, kind="ExternalInput")
    wk = nc.dram_tensor("wk", [D, HPC * DK], BF16, kind="ExternalInput")
    wv = nc.dram_tensor("wv", [D, HPC * DK], BF16, kind="ExternalInput")
    pw = nc.dram_tensor("pw", [D, D], BF16, kind="ExternalInput")
    qrows = nc.dram_tensor("qrows", [ROWS, D], F32, kind="ExternalInput")
    pb = nc.dram_tensor("pb", [1, D], F32, kind="ExternalInput")
    lg = nc.dram_tensor("lg", [1, D], F32, kind="ExternalInput")
    lb = nc.dram_tensor("lb", [1, D], F32, kind="ExternalInput")

    # attn output in transposed (head, m, q) layout; host transposes to (q, m)
    attn_out = nc.dram_tensor("attn_out", [HPC, L, L], F32, kind="ExternalOutput")
    y_out = nc.dram_tensor("y_out", [ROWS, D], F32, kind="ExternalOutput")

    with tile.TileContext(nc) as tc:
        _build_tile_program(nc, tc, qT, kT, vT, wq, wk, wv, pw, qrows,
                            pb, lg, lb, attn_out, y_out)
    nc.compile()
    return nc


def _build_tile_program(nc, tc, qT, kT, vT, wq, wk, wv, pw, qrows,
                        pb, lg, lb, attn_out, y_out):
    from contextlib import ExitStack

    with ExitStack() as ctx:
        const = ctx.enter_context(tc.tile_pool(name="const", bufs=1))
        persist = ctx.enter_context(tc.tile_pool(name="persist", bufs=1))
        dram = ctx.enter_context(tc.tile_pool(name="dram", bufs=1, space="DRAM"))

        identity = const.tile([P, P], F32)
        make_identity(nc, identity[:])
        identity_r = const.tile([P, P], F32R)
        nc.vector.tensor_copy(identity_r[:], identity[:])

        # persistent per-head projections
        qhT = persist.tile([P, HPC, L], F32R)   # (dk, head, L)
        khT = persist.tile([P, HPC, L], F32R)   # (dk, head, L)
        vh = persist.tile([P, NMC, HPC * DK], F32R)   # (m_in_chunk, mchunk, h*dv)
        outT = persist.tile([P, HPC, L], BF16)  # (dv, head, q)

        # per-head A2A buffers (split so head 0's exchange overlaps head 1's
        # attention): rank j's shard of a2a_in_h goes to rank j; after the
        # exchange a2a_out_h[i] holds core i's head-h output for our q-slice.
        a2a_in_h = [dram.tile([NCORES, P, ROWS], BF16, name=f"a2a_in{h}", tag=f"a2a_in{h}")
                    for h in range(HPC)]
        a2a_out_h = [dram.tile([NCORES, P, ROWS], BF16, name=f"a2a_out{h}", tag=f"a2a_out{h}")
                     for h in range(HPC)]

        # ---------------- Phase 1: projections ----------------
        # qhT/khT/vhT: out = W[dc].T @ actT[dc] accumulated over dc.
        with tc.tile_pool(name="vhT_pool", bufs=1) as vhT_pool, \
             tc.tile_pool(name="acts", bufs=3) as acts, \
             tc.tile_pool(name="wts", bufs=3) as wts:
            vhT = vhT_pool.tile([P, HPC, L], F32)
            with tc.tile_pool(name="ps1", bufs=2, space="PSUM") as ps1:
                for actT_d, w_d, dstT in ((qT, wq, qhT), (kT, wk, khT), (vT, wv, vhT)):
                    ps_h = [ps1.tile([P, L], F32, name="ps1t", tag="ps1")
                            for _ in range(HPC)]
                    for dc in range(NDC):
                        at = acts.tile([P, L], BF16)
                        nc.sync.dma_start(at[:], actT_d[dc * P:(dc + 1) * P, :])
                        wt = wts.tile([P, HPC * DK], BF16)
                        nc.sync.dma_start(wt[:], w_d[dc * P:(dc + 1) * P, :])
                        for h in range(HPC):
                            for j in range(NQ4):
                                nc.tensor.matmul(
                                    ps_h[h][:, j * 512:(j + 1) * 512],
                                    wt[:, h * DK:(h + 1) * DK],
                                    at[:, j * 512:(j + 1) * 512],
                                    start=(dc == 0), stop=(dc == NDC - 1),
                                )
                    for h in range(HPC):
                        nc.vector.tensor_copy(dstT[:, h, :], ps_h[h][:])

            # vh = transpose(vhT): (dv, m) -> (m, dv) blocks
            with tc.tile_pool(name="psTr1", bufs=2, space="PSUM") as psTr1:
                for h in range(HPC):
                    for mc in range(NMC):
                        tp = psTr1.tile([P, P], F32)
                        nc.tensor.transpose(
                            tp[:], vhT[:, h, mc * P:(mc + 1) * P], identity[:])
                        nc.vector.tensor_copy(vh[:, mc, h * DK:(h + 1) * DK], tp[:])

        # ---------------- Phase 2+3: scores/softmax/attn, attn @ V ----------
        # per (head, mchunk):
        #   scoresT (m, q) = khT[:, m]T @ qhT  (single K=128 matmul per tile)
        #   attn_raw = exp(scoresT / temper)  with per-row (per-m) sums
        #   attn = attn_raw * (1/sum)         [softmax over q for fixed m]
        #   outT += vh[mc].T @ attn           (accumulate over mchunks)
        #   attn.T blocks -> (q, m) layout -> DRAM
        pwp = ctx.enter_context(tc.tile_pool(name="pwp", bufs=1))
        pw_sb = pwp.tile([P, NDC, D], BF16)
        # preload proj_w during the attention phase (input DMA queues are idle
        # there; proj then starts with weights already resident)
        nc.sync.dma_start(pw_sb[:], pw.rearrange("(hc p) d -> p hc d", p=P))

        with tc.tile_pool(name="attn_sb", bufs=2) as attn_sb_pool, \
             tc.tile_pool(name="vhs_sb", bufs=3) as vhs_pool, \
             tc.tile_pool(name="ssum", bufs=4) as ssum_pool, \
             tc.tile_pool(name="psS", bufs=4, space="PSUM") as psS_pool, \
             tc.tile_pool(name="psO", bufs=1, space="PSUM") as psO_pool:
            for h in range(HPC):
                ps_o = psO_pool.tile([P, L], F32, name="ps_o", tag="ps_o")
                for mc in range(NMC):
                    attn_raw = attn_sb_pool.tile([P, L], F32R, tag="attn_raw")
                    attn_s = attn_sb_pool.tile([P, L], F32, tag="attn_s")
                    ssum4 = ssum_pool.tile([P, NQ4], F32, tag="ssum4")
                    for j in range(NQ4):
                        ps_s = psS_pool.tile([P, 512], F32, name="ps_s", tag="ps_s")
                        nc.tensor.matmul(
                            ps_s[:],
                            khT[:, h, mc * P:(mc + 1) * P],
                            qhT[:, h, j * 512:(j + 1) * 512],
                            start=True, stop=True,
                        )
                        nc.scalar.activation(
                            out=attn_raw[:, j * 512:(j + 1) * 512],
                            in_=ps_s[:],
                            func=mybir.ActivationFunctionType.Exp,
                            scale=INV_TEMPER,
                            accum_out=ssum4[:, j:j + 1],
                        )
                    stot = ssum_pool.tile([P, 1], F32, tag="stot")
                    rs = ssum_pool.tile([P, 1], F32, tag="rs")
                    nc.vector.tensor_reduce(
                        stot[:], ssum4[:], axis=mybir.AxisListType.X,
                        op=mybir.AluOpType.add)
                    nc.vector.reciprocal(rs[:], stot[:])

                    # fold 1/sum into the tiny vh slice so the attn@V matmuls
                    # only wait on the (cheap) sums, not the big normalize
                    vhs = vhs_pool.tile([P, DK], F32R, tag="vhs")
                    nc.vector.tensor_scalar_mul(
                        vhs[:], vh[:, mc, h * DK:(h + 1) * DK], rs[:])
                    for j in range(NQ4):
                        nc.tensor.matmul(
                            ps_o[:, j * 512:(j + 1) * 512],
                            vhs[:],
                            attn_raw[:, j * 512:(j + 1) * 512],
                            start=(mc == 0), stop=(mc == NMC - 1),
                        )

                    # normalized attention row block (m, q) -> DRAM (host
                    # transposes to (q, m) during unshard)
                    nc.vector.tensor_scalar_mul(attn_s[:], attn_raw[:], rs[:])
                    nc.scalar.dma_start(
                        attn_out[h, mc * P:(mc + 1) * P, :], attn_s[:])
                nc.vector.tensor_copy(outT[:, h, :], ps_o[:])
                # exchange this head's output now -- head 0's AllToAll overlaps
                # head 1's attention
                nc.scalar.dma_start(
                    a2a_in_h[h].rearrange("j p c -> p j c"),
                    outT[:, h, :].rearrange("p (j c) -> p j c", c=ROWS),
                )
                nc.gpsimd.collective_compute(
                    "AllToAll",
                    mybir.AluOpType.bypass,
                    replica_groups=[list(range(NCORES))],
                    ins=[a2a_in_h[h].opt()],
                    outs=[a2a_out_h[h].opt()],
                )

        # ---------------- Phase 5: output projection -------------------------
        # proj[q, d] = sum_hdv out_cat[q, hdv] * pw[hdv, d] for q in our rows
        with tc.tile_pool(name="a2t", bufs=2) as a2tp, \
             tc.tile_pool(name="psY", bufs=2, space="PSUM") as psY_pool, \
             tc.tile_pool(name="ln", bufs=1) as lnp, \
             tc.tile_pool(name="lnt", bufs=2) as lntp:
            ps_y = [psY_pool.tile([P, D], F32, name="psYt", tag="psY") for _ in range(ROWS // P)]
            a2sb = [a2tp.tile([P, NCORES, ROWS], BF16, name=f"a2sb{h}", tag="a2sb")
                    for h in range(HPC)]
            for h in range(HPC):
                nc.sync.dma_start(a2sb[h][:], a2a_out_h[h].rearrange("i p c -> p i c"))
            for hc in range(NDC):
                i, h = hc // HPC, hc % HPC
                for qc in range(ROWS // P):
                    for j in range(NQ4):
                        nc.tensor.matmul(
                            ps_y[qc][:, j * 512:(j + 1) * 512],
                            a2sb[h][:, i, qc * P:(qc + 1) * P],
                            pw_sb[:, 2 * i + h, j * 512:(j + 1) * 512],
                            start=(hc == 0), stop=(hc == NDC - 1),
                        )

            # ------------- Phase 6: +bias +residual, LayerNorm, store -------
            pbb = lnp.tile([P, D], F32)
            lgb = lnp.tile([P, D], F32)
            lbb = lnp.tile([P, D], F32)
            for t, d in ((pbb, pb), (lgb, lg), (lbb, lb)):
                bcast = bass.AP(tensor=d.ap().tensor, offset=0,
                                ap=[[0, P], [1, D]])
                nc.gpsimd.dma_start(t[:], bcast)
            eps_t = lnp.tile([P, 1], F32)
            nc.vector.memset(eps_t[:], LN_EPS)

            fmax = nc.vector.BN_STATS_FMAX
            nsub = D // fmax
            for qc in range(ROWS // P):
                qr_t = lntp.tile([P, D], F32, tag="qr")
                nc.sync.dma_start(qr_t[:], qrows[qc * P:(qc + 1) * P, :])
                y0 = lntp.tile([P, D], F32, tag="y0")
                nc.vector.tensor_add(y0[:], ps_y[qc][:], qr_t[:])
                nc.vector.tensor_add(y0[:], y0[:], pbb[:])

                stats = lntp.tile([P, nsub, nc.vector.BN_STATS_DIM], F32, tag="st")
                for sg in range(nsub):
                    nc.vector.bn_stats(
                        out=stats[:, sg, :],
                        in_=y0[:, sg * fmax:(sg + 1) * fmax])
                mv = lntp.tile([P, nc.vector.BN_AGGR_DIM], F32, tag="mv")
                nc.vector.bn_aggr(out=mv[:], in_=stats[:])
                mean = mv[:, 0:1]
                rstd = mv[:, 1:2]
                nc.scalar.activation(
                    out=rstd, in_=rstd,
                    func=mybir.ActivationFunctionType.Sqrt,
                    bias=eps_t[:], scale=1.0)
                nc.vector.reciprocal(out=rstd, in_=rstd)
                nc.vector.tensor_scalar(
                    out=y0[:], in0=y0[:],
                    scalar1=mean, scalar2=rstd,
                    op0=mybir.AluOpType.subtract, op1=mybir.AluOpType.mult)
                nc.vector.tensor_mul(y0[:], y0[:], lgb[:])
                nc.vector.tensor_add(y0[:], y0[:], lbb[:])
                nc.sync.dma_start(y_out[qc * P:(qc + 1) * P, :], y0[:])


_NC_CACHE = None


def _get_nc():
    global _NC_CACHE
    if _NC_CACHE is None:
        _NC_CACHE = build_nc()
    return _NC_CACHE


def shard_inputs(q, k, v, Wq, Wk, Wv, proj_w, proj_b, ln_g, ln_b):
    import ml_dtypes
    f = np.float32
    bf = ml_dtypes.bfloat16
    qs, ks, vs = (np.asarray(x, f)[0] for x in (q, k, v))
    qT = np.ascontiguousarray(qs.T.astype(bf))
    kT = np.ascontiguousarray(ks.T.astype(bf))
    vT = np.ascontiguousarray(vs.T.astype(bf))
    pw = np.ascontiguousarray(np.asarray(proj_w, f).astype(bf))
    pb = np.ascontiguousarray(np.asarray(proj_b, f).reshape(1, D))
    lg = np.ascontiguousarray(np.asarray(ln_g, f).reshape(1, D))
    lb = np.ascontiguousarray(np.asarray(ln_b, f).reshape(1, D))
    Wq, Wk, Wv = (np.asarray(x, f) for x in (Wq, Wk, Wv))

    in_maps = []
    for c in range(NCORES):
        h0 = HPC * c
        in_maps.append({
            "qT": qT, "kT": kT, "vT": vT,
            "wq": np.ascontiguousarray(np.concatenate([Wq[h0 + i] for i in range(HPC)], axis=1).astype(bf)),
            "wk": np.ascontiguousarray(np.concatenate([Wk[h0 + i] for i in range(HPC)], axis=1).astype(bf)),
            "wv": np.ascontiguousarray(np.concatenate([Wv[h0 + i] for i in range(HPC)], axis=1).astype(bf)),
            "pw": pw,
            "qrows": np.ascontiguousarray(qs[ROWS * c:ROWS * (c + 1)]),
            "pb": pb, "lg": lg, "lb": lb,
        })
    return in_maps


def assemble_outputs(results):
    y = np.concatenate([results[c]["y_out"] for c in range(NCORES)], axis=0)[None]
    attn = np.empty((H, L, L), np.float32)
    for c in range(NCORES):
        a = results[c]["attn_out"]          # (HPC, m, q) transposed layout
        for h in range(HPC):
            np.copyto(attn[HPC * c + h], a[h].T)
    return y.astype(np.float32, copy=False), attn


def run_sharded(in_maps, trace=False):
    nc = _get_nc()
    res = run_bass_kernel_spmd(nc, in_maps, core_ids=list(range(NCORES)),
                               trace=trace)
    return res


def kernel(**inputs):
    in_maps = shard_inputs(**inputs)
    res = run_sharded(in_maps, trace=False)
    return assemble_outputs(res.results)
